# revision 8
# baseline (speedup 1.0000x reference)
"""MoE feed-forward (top-2 routing + shared expert) on 8 Trainium2 cores.

Strategy (expert parallel):
  - Host computes the router (tiny [T,D]@[D,E] matmul), top-2 expert ids and
    renormalized gates, then dispatches each expert's tokens (transposed,
    capacity-padded) to the core that owns that expert's weights.
  - Core e computes  ye = (silu(xe@w1_e) * (xe@w3_e)) @ w2_e, row-scaled by the
    gate, plus a 1/8 token-slice of the always-active shared expert.
  - Host scatter-adds routed outputs into the shared-expert output.

On-device matmuls use float32r (fp32 data truncated to fp22 multiplies,
fp32 accumulation) which runs the PE at full rate for free dims >= 256.
"""

import numpy as np

import concourse.bass as bass
import concourse.mybir as mybir
import concourse.tile as tile
from concourse import bacc
from concourse.bass_utils import run_bass_kernel_spmd

P = 128
N_CORES = 8
F32 = mybir.dt.float32
F32R = mybir.dt.float32r
AF = mybir.ActivationFunctionType

# h-tiles of w1/w3 fetched per DMA (bigger transfers, fewer descriptors)
H_BLOCK = 2


def _swiglu_block(
    tc, pools, xT_ap, n_rows, w1_ap, w3_ap, w2_ap, out_ap, ge_ap, use_silu=True
):
    """Emit one SwiGLU y = (silu(x@w1) * (x@w3)) @ w2 over n_rows tokens.

    xT_ap: [D, n_rows] (transposed activations), w1/w3: [D, H], w2: [H, D],
    out_ap: [n_rows, D]. If ge_ap ([n_rows, 1]) is given, output rows are
    scaled by it.
    """
    nc = tc.nc
    D = w1_ap.shape[0]
    H = w1_ap.shape[1]
    KD = D // P
    KH = H // P
    ND = D // 512  # output free-dim tiles

    xpool, w2pool, wpool, gpool, spool, opool, gepool, pp1, pp3, ppo = pools

    # Resident tiles for this block
    xt = xpool.tile([P, KD, n_rows], F32R, tag="xT")
    nc.sync.dma_start(xt[:], xT_ap.rearrange("(k p) n -> p k n", p=P))
    w2t = w2pool.tile([P, KH, D], F32R, tag="w2res")
    nc.sync.dma_start(w2t[:], w2_ap.rearrange("(k p) d -> p k d", p=P))
    if ge_ap is not None:
        get_ = gepool.tile([P, n_rows // P], F32, tag="ge")
        nc.sync.dma_start(get_[:], ge_ap.rearrange("(c p) one -> p (c one)", p=P))

    w1r = w1_ap.rearrange("(k p) h -> p k h", p=P)
    w3r = w3_ap.rearrange("(k p) h -> p k h", p=P)

    c0 = 0
    while c0 < n_rows:
        cw = min(512, n_rows - c0)  # chunk width (token columns)

        # ---- phase 1: gT[h, c] = silu(h1T) * h3T for this chunk ----
        gt = gpool.tile([P, KH, 512], F32R, tag="gT")
        for hb in range(KH // H_BLOCK):
            w1t = wpool.tile([P, KD, H_BLOCK * P], F32R, tag="w1t")
            nc.sync.dma_start(
                w1t[:], w1r[:, :, hb * H_BLOCK * P : (hb + 1) * H_BLOCK * P]
            )
            w3t = wpool.tile([P, KD, H_BLOCK * P], F32R, tag="w3t")
            nc.sync.dma_start(
                w3t[:], w3r[:, :, hb * H_BLOCK * P : (hb + 1) * H_BLOCK * P]
            )
            for hi in range(H_BLOCK):
                h = hb * H_BLOCK + hi
                p1 = pp1.tile([P, 512], F32, tag="p1", name="p1")[:, :cw]
                p3 = pp3.tile([P, 512], F32, tag="p3", name="p3")[:, :cw]
                for k in range(KD):
                    nc.tensor.matmul(
                        p1,
                        w1t[:, k, hi * P : (hi + 1) * P],
                        xt[:, k, c0 : c0 + cw],
                        start=(k == 0),
                        stop=(k == KD - 1),
                    )
                for k in range(KD):
                    nc.tensor.matmul(
                        p3,
                        w3t[:, k, hi * P : (hi + 1) * P],
                        xt[:, k, c0 : c0 + cw],
                        start=(k == 0),
                        stop=(k == KD - 1),
                    )
                s1 = spool.tile([P, 512], F32, tag="s1", name="s1")[:, :cw]
                if use_silu:
                    nc.scalar.activation(s1, p1, AF.Silu)
                    nc.vector.tensor_mul(gt[:, h, :cw], s1, p3)
                else:  # silu(a) = a * sigmoid(a); CoreSim has no Silu table
                    nc.scalar.activation(s1, p1, AF.Sigmoid)
                    nc.vector.tensor_mul(gt[:, h, :cw], p1, p3)
                    nc.vector.tensor_mul(gt[:, h, :cw], gt[:, h, :cw], s1)

        # ---- phase 2: out rows = gT.T @ w2 (accumulate over H) ----
        for ct in range(cw // P):
            ot = opool.tile([P, D], F32, tag="ot")
            for dn in range(ND):
                po = ppo.tile([P, 512], F32, tag="po")
                for kh in range(KH):
                    nc.tensor.matmul(
                        po,
                        gt[:, kh, ct * P : (ct + 1) * P],
                        w2t[:, kh, dn * 512 : (dn + 1) * 512],
                        start=(kh == 0),
                        stop=(kh == KH - 1),
                    )
                if ge_ap is not None:
                    nc.vector.tensor_scalar_mul(
                        ot[:, dn * 512 : (dn + 1) * 512],
                        po,
                        get_[:, c0 // P + ct : c0 // P + ct + 1],
                    )
                else:
                    nc.vector.tensor_copy(ot[:, dn * 512 : (dn + 1) * 512], po)
            nc.sync.dma_start(out_ap[c0 + ct * P : c0 + (ct + 1) * P, :], ot[:])
        c0 += cw


def build_moe_program(D, H, C, S, use_silu=True):
    """SPMD program: routed expert over C capacity rows + shared expert over
    S token-slice rows. Same NEFF on all 8 cores, per-core input data."""
    nc = bacc.Bacc(
        "TRN2", target_bir_lowering=False, debug=False, num_devices=N_CORES
    )

    def din(name, shape, dt=F32):
        return nc.dram_tensor(name, shape, dt, kind="ExternalInput").ap()

    def dout(name, shape):
        return nc.dram_tensor(name, shape, F32, kind="ExternalOutput").ap()

    xeT = din("xeT", [D, C], F32R)
    ge = din("ge", [C, 1])
    xsT = din("xsT", [D, S], F32R)
    w1 = din("w1", [D, H], F32R)
    w3 = din("w3", [D, H], F32R)
    w2 = din("w2", [H, D], F32R)
    sw1 = din("sw1", [D, H], F32R)
    sw3 = din("sw3", [D, H], F32R)
    sw2 = din("sw2", [H, D], F32R)
    ye = dout("ye", [C, D])
    se = dout("se", [S, D])

    with tile.TileContext(nc) as tc:
        from contextlib import ExitStack

        with ExitStack() as ctx:
            pools = (
                ctx.enter_context(tc.tile_pool(name="xT", bufs=1)),
                ctx.enter_context(tc.tile_pool(name="w2res", bufs=1)),
                ctx.enter_context(tc.tile_pool(name="wstream", bufs=2)),
                ctx.enter_context(tc.tile_pool(name="gT", bufs=1)),
                ctx.enter_context(tc.tile_pool(name="stemp", bufs=3)),
                ctx.enter_context(tc.tile_pool(name="otile", bufs=3)),
                ctx.enter_context(tc.tile_pool(name="gate", bufs=1)),
                ctx.enter_context(tc.tile_pool(name="ps1", bufs=2, space="PSUM")),
                ctx.enter_context(tc.tile_pool(name="ps3", bufs=2, space="PSUM")),
                ctx.enter_context(tc.tile_pool(name="pso", bufs=2, space="PSUM")),
            )
            _swiglu_block(tc, pools, xeT, C, w1, w3, w2, ye, ge, use_silu)
            _swiglu_block(tc, pools, xsT, S, sw1, sw3, sw2, se, None, use_silu)

    nc.compile()
    return nc


_PROGRAM_CACHE = {}
LAST_RESULTS = None  # BassKernelResults of the most recent device run (for test.py)


def _get_program(D, H, C, S):
    key = (D, H, C, S)
    if key not in _PROGRAM_CACHE:
        _PROGRAM_CACHE[key] = build_moe_program(D, H, C, S)
    return _PROGRAM_CACHE[key]


def _route(xf, w_router):
    """Top-2 routing identical (up to fp rounding) to the jax reference."""
    logits = xf @ w_router.astype(np.float32)  # [T, E]
    # softmax is monotone: top-2 of probs == top-2 of logits, stable ties
    top2 = np.argsort(-logits, axis=1, kind="stable")[:, :2]  # [T, 2]
    lv = np.take_along_axis(logits, top2, axis=1)
    ev = np.exp(lv - lv[:, 0:1])
    gates = ev / ev.sum(axis=1, keepdims=True)  # [T, 2] renormalized
    return top2, gates


def kernel(x, w_router, w1, w3, w2, sw1, sw3, sw2):
    B, SEQ, D = x.shape
    T = B * SEQ
    E, _, H = w1.shape
    assert E == N_CORES
    S = T // N_CORES

    x = np.asarray(x, dtype=np.float32)
    xf = np.ascontiguousarray(x.reshape(T, D))
    top2, gates = _route(xf, np.asarray(w_router, np.float32))

    # per-expert token lists + gate values
    flat_e = top2.ravel()  # slot 2t, 2t+1 -> token t
    flat_g = gates.ravel().astype(np.float32)
    order = np.argsort(flat_e, kind="stable")
    sorted_e = flat_e[order]
    starts = np.searchsorted(sorted_e, np.arange(E + 1))
    tok_by_e = [order[starts[e] : starts[e + 1]] >> 1 for e in range(E)]
    gate_by_e = [flat_g[order[starts[e] : starts[e + 1]]] for e in range(E)]
    counts = np.diff(starts)

    # capacity: fixed floor so the compiled program is reused across calls
    C = max(1280, (int(counts.max()) + 127) // 128 * 128)

    nc = _get_program(D, H, C, S)

    w1 = np.asarray(w1, np.float32)
    w3 = np.asarray(w3, np.float32)
    w2 = np.asarray(w2, np.float32)
    sw1 = np.ascontiguousarray(np.asarray(sw1, np.float32))
    sw3 = np.ascontiguousarray(np.asarray(sw3, np.float32))
    sw2 = np.ascontiguousarray(np.asarray(sw2, np.float32))

    in_maps = []
    for e in range(E):
        n_e = int(counts[e])
        xeT = np.zeros((D, C), np.float32)
        xeT[:, :n_e] = xf[tok_by_e[e]].T
        ge = np.zeros((C, 1), np.float32)
        ge[:n_e, 0] = gate_by_e[e]
        xsT = np.ascontiguousarray(xf[e * S : (e + 1) * S].T)
        in_maps.append(
            {
                "xeT": xeT,
                "ge": ge,
                "xsT": xsT,
                "w1": np.ascontiguousarray(w1[e]),
                "w3": np.ascontiguousarray(w3[e]),
                "w2": np.ascontiguousarray(w2[e]),
                "sw1": sw1,
                "sw3": sw3,
                "sw2": sw2,
            }
        )

    global LAST_RESULTS
    LAST_RESULTS = run_bass_kernel_spmd(nc, in_maps, core_ids=list(range(N_CORES)))
    res = LAST_RESULTS.results

    out = np.empty((T, D), np.float32)
    for c in range(N_CORES):
        out[c * S : (c + 1) * S] = res[c]["se"]
    for e in range(E):
        n_e = int(counts[e])
        if n_e:
            out[tok_by_e[e]] += res[e]["ye"][:n_e]
    return out.reshape(B, SEQ, D)


# revision 10
# speedup vs baseline: 1.1241x; 1.1241x over previous
"""MoE feed-forward (top-2 routing + shared expert) on 8 Trainium2 cores.

Strategy (expert parallel):
  - Host computes the router (tiny [T,D]@[D,E] matmul), top-2 expert ids and
    renormalized gates, then dispatches each expert's tokens (transposed,
    capacity-padded) to the core that owns that expert's weights.
  - Core e computes  ye = (silu(xe@w1_e) * (xe@w3_e)) @ w2_e, row-scaled by the
    gate, plus a 1/8 token-slice of the always-active shared expert.
  - Host scatter-adds routed outputs into the shared-expert output.

On-device matmuls use float32r (fp32 data with fp22 multiplies, fp32
accumulation) which runs the PE at full rate for free dims >= 256.
"""

import numpy as np

import concourse.bass as bass
import concourse.mybir as mybir
import concourse.tile as tile
from concourse import bacc
from concourse.bass_utils import run_bass_kernel_spmd

P = 128
N_CORES = 8
F32 = mybir.dt.float32
F32R = mybir.dt.float32r
AF = mybir.ActivationFunctionType

# h-tiles of w1/w3 fetched per DMA (bigger transfers, fewer descriptors)
H_BLOCK = 2


def _chunk_widths(n):
    """Split n (multiple of 128) into widths of 256..512 (multiples of 128):
    float32r matmuls run at full PE rate only for free dim >= 256."""
    assert n % P == 0
    if n < 2 * P:
        return [n]
    widths = []
    while n > 0:
        if n >= 640:
            widths.append(512)
            n -= 512
        elif n == 512:
            widths.append(512)
            n = 0
        else:  # 256, 384
            widths.append(n)
            n = 0
    return widths


def _swiglu_block(
    tc, pools, xT_ap, n_rows, w1_ap, w3_ap, w2_ap, out_ap, ge_ap, use_silu=True
):
    """Emit one SwiGLU y = (silu(x@w1) * (x@w3)) @ w2 over n_rows tokens.

    xT_ap: [D, n_rows] (transposed activations), w1/w3: [D, H], w2: [H, D],
    out_ap: [n_rows, D]. If ge_ap ([n_rows, 1]) is given, output rows are
    scaled by it.
    """
    nc = tc.nc
    D = w1_ap.shape[0]
    H = w1_ap.shape[1]
    KD = D // P
    KH = H // P
    ND = D // 512  # output free-dim tiles

    xpool, w2pool, wpool, gpool, spool, opool, gepool, pp1, pp3, ppo = pools

    if ge_ap is not None:
        get_ = gepool.tile([P, n_rows // P], F32, tag="ge")
        nc.sync.dma_start(get_[:], ge_ap.rearrange("(c p) one -> p (c one)", p=P))

    xr = xT_ap.rearrange("(k p) n -> p k n", p=P)
    w1r = w1_ap.rearrange("(k p) h -> p k h", p=P)
    w3r = w3_ap.rearrange("(k p) h -> p k h", p=P)
    w2r = w2_ap.rearrange("(k p) d -> p k d", p=P)
    w2t = None  # loaded lazily so startup DMAs prioritize phase-1 operands

    c0 = 0
    for cw in _chunk_widths(n_rows):
        # per-chunk activation slice (double-buffered: next chunk prefetches)
        xt = xpool.tile([P, KD, 512], F32R, tag="xT", name="xt")[:, :, :cw]
        nc.sync.dma_start(xt[:], xr[:, :, c0 : c0 + cw])

        # ---- phase 1: gT[h, c] = silu(h1T) * h3T for this chunk ----
        gt = gpool.tile([P, KH, 512], F32R, tag="gT")
        for hb in range(KH // H_BLOCK):
            w1t = wpool.tile([P, KD, H_BLOCK * P], F32R, tag="w1t")
            nc.sync.dma_start(
                w1t[:], w1r[:, :, hb * H_BLOCK * P : (hb + 1) * H_BLOCK * P]
            )
            w3t = wpool.tile([P, KD, H_BLOCK * P], F32R, tag="w3t")
            nc.sync.dma_start(
                w3t[:], w3r[:, :, hb * H_BLOCK * P : (hb + 1) * H_BLOCK * P]
            )
            for hi in range(H_BLOCK):
                h = hb * H_BLOCK + hi
                p1 = pp1.tile([P, 512], F32, tag="p1", name="p1")[:, :cw]
                p3 = pp3.tile([P, 512], F32, tag="p3", name="p3")[:, :cw]
                for k in range(KD):
                    nc.tensor.matmul(
                        p1,
                        w1t[:, k, hi * P : (hi + 1) * P],
                        xt[:, k, :],
                        start=(k == 0),
                        stop=(k == KD - 1),
                    )
                for k in range(KD):
                    nc.tensor.matmul(
                        p3,
                        w3t[:, k, hi * P : (hi + 1) * P],
                        xt[:, k, :],
                        start=(k == 0),
                        stop=(k == KD - 1),
                    )
                s1 = spool.tile([P, 512], F32, tag="s1", name="s1")[:, :cw]
                if use_silu:
                    nc.scalar.activation(s1, p1, AF.Silu)
                    nc.vector.tensor_mul(gt[:, h, :cw], s1, p3)
                else:  # silu(a) = a * sigmoid(a); CoreSim has no Silu table
                    nc.scalar.activation(s1, p1, AF.Sigmoid)
                    nc.vector.tensor_mul(gt[:, h, :cw], p1, p3)
                    nc.vector.tensor_mul(gt[:, h, :cw], gt[:, h, :cw], s1)

        if w2t is None:
            # emitted after the first phase-1 so startup DMAs aren't stuck
            # behind this 8 MB transfer; needed only once phase 2 begins
            w2t = w2pool.tile([P, KH, D], F32R, tag="w2res", name="w2t")
            half = KH // 2
            nc.sync.dma_start(w2t[:, :half, :], w2r[:, :half, :])
            nc.sync.dma_start(w2t[:, half:, :], w2r[:, half:, :])

        # ---- phase 2: out rows = gT.T @ w2 (accumulate over H) ----
        for ct in range(cw // P):
            ot = opool.tile([P, D], F32, tag="ot")
            for dn in range(ND):
                po = ppo.tile([P, 512], F32, tag="po")
                for kh in range(KH):
                    nc.tensor.matmul(
                        po,
                        gt[:, kh, ct * P : (ct + 1) * P],
                        w2t[:, kh, dn * 512 : (dn + 1) * 512],
                        start=(kh == 0),
                        stop=(kh == KH - 1),
                    )
                if ge_ap is not None:
                    nc.vector.tensor_scalar_mul(
                        ot[:, dn * 512 : (dn + 1) * 512],
                        po,
                        get_[:, c0 // P + ct : c0 // P + ct + 1],
                    )
                else:
                    nc.vector.tensor_copy(ot[:, dn * 512 : (dn + 1) * 512], po)
            nc.sync.dma_start(out_ap[c0 + ct * P : c0 + (ct + 1) * P, :], ot[:])
        c0 += cw


def build_moe_program(D, H, C, S, use_silu=True):
    """SPMD program: routed expert over C capacity rows + shared expert over
    S token-slice rows. Same NEFF on all 8 cores, per-core input data."""
    nc = bacc.Bacc(
        "TRN2", target_bir_lowering=False, debug=False, num_devices=N_CORES
    )

    def din(name, shape, dt=F32):
        return nc.dram_tensor(name, shape, dt, kind="ExternalInput").ap()

    def dout(name, shape):
        return nc.dram_tensor(name, shape, F32, kind="ExternalOutput").ap()

    xeT = din("xeT", [D, C], F32R)
    ge = din("ge", [C, 1])
    xsT = din("xsT", [D, S], F32R)
    w1 = din("w1", [D, H], F32R)
    w3 = din("w3", [D, H], F32R)
    w2 = din("w2", [H, D], F32R)
    sw1 = din("sw1", [D, H], F32R)
    sw3 = din("sw3", [D, H], F32R)
    sw2 = din("sw2", [H, D], F32R)
    ye = dout("ye", [C, D])
    se = dout("se", [S, D])

    with tile.TileContext(nc) as tc:
        from contextlib import ExitStack

        with ExitStack() as ctx:
            pools = (
                ctx.enter_context(tc.tile_pool(name="xT", bufs=2)),
                ctx.enter_context(tc.tile_pool(name="w2res", bufs=1)),
                ctx.enter_context(tc.tile_pool(name="wstream", bufs=2)),
                ctx.enter_context(tc.tile_pool(name="gT", bufs=2)),
                ctx.enter_context(tc.tile_pool(name="stemp", bufs=2)),
                ctx.enter_context(tc.tile_pool(name="otile", bufs=2)),
                ctx.enter_context(tc.tile_pool(name="gate", bufs=1)),
                ctx.enter_context(tc.tile_pool(name="ps1", bufs=2, space="PSUM")),
                ctx.enter_context(tc.tile_pool(name="ps3", bufs=2, space="PSUM")),
                ctx.enter_context(tc.tile_pool(name="pso", bufs=2, space="PSUM")),
            )
            _swiglu_block(tc, pools, xeT, C, w1, w3, w2, ye, ge, use_silu)
            _swiglu_block(tc, pools, xsT, S, sw1, sw3, sw2, se, None, use_silu)

    nc.compile()
    return nc


_PROGRAM_CACHE = {}
LAST_RESULTS = None  # BassKernelResults of the most recent device run (for test.py)


def _get_program(D, H, C, S):
    key = (D, H, C, S)
    if key not in _PROGRAM_CACHE:
        _PROGRAM_CACHE[key] = build_moe_program(D, H, C, S)
    return _PROGRAM_CACHE[key]


def _route(xf, w_router):
    """Top-2 routing identical (up to fp rounding) to the jax reference."""
    logits = xf @ w_router.astype(np.float32)  # [T, E]
    # softmax is monotone: top-2 of probs == top-2 of logits, stable ties
    top2 = np.argsort(-logits, axis=1, kind="stable")[:, :2]  # [T, 2]
    lv = np.take_along_axis(logits, top2, axis=1)
    ev = np.exp(lv - lv[:, 0:1])
    gates = ev / ev.sum(axis=1, keepdims=True)  # [T, 2] renormalized
    return top2, gates


def kernel(x, w_router, w1, w3, w2, sw1, sw3, sw2):
    B, SEQ, D = x.shape
    T = B * SEQ
    E, _, H = w1.shape
    assert E == N_CORES
    S = T // N_CORES

    x = np.asarray(x, dtype=np.float32)
    xf = np.ascontiguousarray(x.reshape(T, D))
    top2, gates = _route(xf, np.asarray(w_router, np.float32))

    # per-expert token lists + gate values
    flat_e = top2.ravel()  # slot 2t, 2t+1 -> token t
    flat_g = gates.ravel().astype(np.float32)
    order = np.argsort(flat_e, kind="stable")
    sorted_e = flat_e[order]
    starts = np.searchsorted(sorted_e, np.arange(E + 1))
    tok_by_e = [order[starts[e] : starts[e + 1]] >> 1 for e in range(E)]
    gate_by_e = [flat_g[order[starts[e] : starts[e + 1]]] for e in range(E)]
    counts = np.diff(starts)

    # capacity: fixed floor so the compiled program is reused across calls
    C = max(1152, (int(counts.max()) + 127) // 128 * 128)

    nc = _get_program(D, H, C, S)

    w1 = np.asarray(w1, np.float32)
    w3 = np.asarray(w3, np.float32)
    w2 = np.asarray(w2, np.float32)
    sw1 = np.ascontiguousarray(np.asarray(sw1, np.float32))
    sw3 = np.ascontiguousarray(np.asarray(sw3, np.float32))
    sw2 = np.ascontiguousarray(np.asarray(sw2, np.float32))

    in_maps = []
    for e in range(E):
        n_e = int(counts[e])
        xeT = np.zeros((D, C), np.float32)
        xeT[:, :n_e] = xf[tok_by_e[e]].T
        ge = np.zeros((C, 1), np.float32)
        ge[:n_e, 0] = gate_by_e[e]
        xsT = np.ascontiguousarray(xf[e * S : (e + 1) * S].T)
        in_maps.append(
            {
                "xeT": xeT,
                "ge": ge,
                "xsT": xsT,
                "w1": np.ascontiguousarray(w1[e]),
                "w3": np.ascontiguousarray(w3[e]),
                "w2": np.ascontiguousarray(w2[e]),
                "sw1": sw1,
                "sw3": sw3,
                "sw2": sw2,
            }
        )

    global LAST_RESULTS
    LAST_RESULTS = run_bass_kernel_spmd(nc, in_maps, core_ids=list(range(N_CORES)))
    res = LAST_RESULTS.results

    out = np.empty((T, D), np.float32)
    for c in range(N_CORES):
        out[c * S : (c + 1) * S] = res[c]["se"]
    for e in range(E):
        n_e = int(counts[e])
        if n_e:
            out[tok_by_e[e]] += res[e]["ye"][:n_e]
    return out.reshape(B, SEQ, D)
